# revision 13
# baseline (speedup 1.0000x reference)
"""Trainium2 Bass kernel for nn_DualOutputMoE.

Math: the reference collapses the whole MoE into a single [1,1,H] vector:
    acc = sum_e combine[:,e] @ (gelu(x @ W1[e] + b1[e]) @ W2[e] + b2[e])
    out = acc / total_weight
Since combine is applied *linearly* after the gelu, we contract it with the
gelu activations first:
    u_e  = combine[:,e] @ gelu(x @ W1[e] + b1[e])          # [F]
    acc  = sum_e (u_e @ W2[e] + combine[:,e].sum() * b2[e])
which turns the second [T,F]@[F,H] matmul into an [F]@[F,H] matvec.
Only tokens actually routed to expert e (combine[:,e] != 0) contribute, so we
gather those tokens on the host (top-2 of 16 experts -> ~T/8 tokens/expert)
and the device computes a dense [C,H]@[H,F] per expert with C = padded
capacity.

Sharding: expert-parallel, 2 experts per core across 8 cores (E=16). Each
core returns partial accumulator rows; the host sums them.

Device kernel (per core, SPMD over 8 cores), three pipelined stages:

Phase 1 (PE-dense): per f-chunk-PAIR (1024 wide, 2 PSUM banks):
  mm1:  psA[tok128, 0:512 | 512:1024] += xgDR[h].T @ W1DR[h, half]
        (fp8 e4m3 DoubleRow, 4 DR k-tiles of 256)
  gelu: G[128, j, 1024] = gelu(psA / 32)   (ACT, psum->sbuf fp8, j = tt parity)
  cmm:  psB[1, half] += cwDR[tok256].T @ G[tok256, half]
        (fp8 DoubleRow over token-tile PAIRS; odd tail tile plain fp8)
  u     -> [1, F] f32 row -> DRAM -> uT [128, F/128] fp16 (partition transpose)
W2 tiles (fp16) stream in from the start on the DVE DMA ring, all 64 resident
in SBUF by the time phase 2 starts.

Phase 2 (tail): mm2 = u @ W2 per expert, fp16, 4-way COLUMN-TILED: units
(expert, F-half) run concurrently on PE col groups 0/32/64/96 (m=1 each), so
128 N=512 matmuls cost ~1/4 the serial time. psC[hc] is one [128,512] bank
per h-half; unit j accumulates into partition 32j. The last-ready unit (needs
the final pair's u) is deferred a few rounds so earlier units bridge the
u-transpose DMA latency. Host sums partition rows 0/32/64/96.
"""

import sys
import math

if "/opt/trn_rl_repo" not in sys.path:
    sys.path.insert(0, "/opt/trn_rl_repo")

import numpy as np
import ml_dtypes

import concourse.bass as bass
import concourse.tile as tile
from concourse import bacc, mybir
from concourse.bass_utils import run_bass_kernel_spmd

BF16 = ml_dtypes.bfloat16
FP8 = ml_dtypes.float8_e4m3
N_CORES = 8
E = 16
EPC = E // N_CORES  # experts per core
H = 1024
F = 4096
TOP_K = 2
KH2 = H // 256  # 4 fp8-DoubleRow k-tiles (256 contraction per tile)
FT = F // 128  # 32 f-tiles along F
FCP = 4  # 4 f-chunk PAIRS of 1024
HC = H // 512  # 2 h-chunks of 512
W1_SCALE = 32.0  # pre-scale W1 into fp8's normal range; gelu un-scales
N_UNITS = 4  # mm2 column-tiling units: (expert, F-half)

_compiled_cache = {}


def _build(nt: int, has_b1: bool, reps: int = 1):
    """Build + compile the SPMD device program for NT token tiles per expert.

    reps > 1 wraps the whole body in a hardware For_i loop running it that
    many times (used by test.py for wall-clock timing)."""
    key = (nt, has_b1, reps)
    if key in _compiled_cache:
        return _compiled_cache[key]

    C = nt * 128
    NP = (nt + 1) // 2  # token-tile pairs (cmm DoubleRow); last may be single
    nc = bacc.Bacc("TRN2", target_bir_lowering=False, debug=False)
    f32 = mybir.dt.float32
    bf16 = mybir.dt.bfloat16
    fp16 = mybir.dt.float16
    fp8 = mybir.dt.float8e4

    NKT = KH2
    KW = 2 * 1024  # w1 free elems per k-tile per fc-pair (DR interleave)
    DR = mybir.MatmulPerfMode.DoubleRow

    # DoubleRow interleave: h = kt*256 + p*2 + j; fc-pair-major so each
    # pair is one contiguous DMA per k-tile with 2KB lines
    xg_d = nc.dram_tensor("xg", [EPC, KH2, 128, 2, C], fp8, kind="ExternalInput").ap()
    w1_d = nc.dram_tensor(
        "w1", [EPC, FCP, NKT, 128, KW], fp8, kind="ExternalInput"
    ).ap()
    cw_d = nc.dram_tensor("cw", [EPC, 128, 2, 16], fp8, kind="ExternalInput").ap()
    w2_d = nc.dram_tensor("w2", [EPC, F, H], fp16, kind="ExternalInput").ap()
    if has_b1:
        b1_d = nc.dram_tensor("b1", [EPC, F], bf16, kind="ExternalInput").ap()
    acc_d = nc.dram_tensor("acc", [128, H], f32, kind="ExternalOutput").ap()
    u_d = nc.dram_tensor("u_scratch", [EPC, F], fp16).ap()

    with tile.TileContext(nc) as tc:
        with (
            tc.tile_pool(name="xg", bufs=1) as xg_pool,
            tc.tile_pool(name="cw", bufs=1) as cw_pool,
            tc.tile_pool(name="w1", bufs=3) as w1_pool,
            tc.tile_pool(name="w2", bufs=EPC * FT) as w2_pool,
            tc.tile_pool(name="g", bufs=6) as g_pool,
            tc.tile_pool(name="u", bufs=1) as u_pool,
            tc.tile_pool(name="small", bufs=1) as small_pool,
            tc.tile_pool(name="psA", bufs=2, space="PSUM") as psA_pool,
            tc.tile_pool(name="psB", bufs=1, space="PSUM") as psB_pool,
            tc.tile_pool(name="psC", bufs=1, space="PSUM") as psC_pool,
        ):
            xg_sb, cw_sb, b1_sb, u8, uT_b = [], [], [], [], []
            ones_sb = None
            for e in range(EPC):
                xg_sb.append(
                    xg_pool.tile([128, KH2, 2, C], fp8, tag=f"xg{e}", name=f"xg{e}")
                )
                # padded to 16 cols so the DoubleRow Ko step is 16B-aligned
                cw_sb.append(
                    cw_pool.tile([128, 2, 16], fp8, tag=f"cw{e}", name=f"cw{e}")
                )
                u8.append(u_pool.tile([1, F], fp16, tag=f"u{e}", name=f"u{e}"))
                uT_b.append(
                    small_pool.tile([128, FT], fp16, tag=f"uTb{e}", name=f"uTb{e}")
                )

            def load_xg(e, kt):
                nc.sync.dma_start(xg_sb[e][:, kt, :, :], xg_d[e, kt])

            def load_cw(e):
                # host pre-lays [p, j, q] with q padded to 16: contiguous DMA
                nc.sync.dma_start(cw_sb[e][:], cw_d[e])

            if has_b1:
                ones_sb = small_pool.tile([1, 128], bf16, tag="ones", name="ones")
                nc.vector.memset(ones_sb[:], 1.0)
                for e in range(EPC):
                    b1_t = small_pool.tile([1, F], bf16, tag=f"b1{e}", name=f"b1{e}")
                    nc.sync.dma_start(b1_t[:], b1_d[e : e + 1, :])
                    b1_sb.append(b1_t)

            def w2_prefetch(e, ft):
                # ACT HWDGE ring: decoupled from the SP ring feeding w1/xg
                w2_t = w2_pool.tile([128, HC, 512], fp16, name="w2t")
                nc.scalar.dma_start(
                    w2_t[:],
                    w2_d[e, ft * 128 : (ft + 1) * 128, :].rearrange(
                        "p (h n) -> p h n", h=HC
                    ),
                )
                return w2_t

            def load_w1(e, fp, split=False):
                w1_t = w1_pool.tile([128, NKT, KW], fp8, name="w1t")
                if split:  # per-k-tile DMAs: first mm waits on one tile only
                    for kt in range(NKT):
                        nc.sync.dma_start(w1_t[:, kt, :], w1_d[e, fp, kt])
                else:
                    nc.sync.dma_start(w1_t[:], w1_d[e, fp].rearrange("k p n -> p k n"))
                return w1_t

            # software-pipelined emission state
            cmm_q = []  # (e, fp, q, g_pair, is_pair)
            psB_cur = {}
            w2_tiles = {}

            def pop_cmm():
                if not cmm_q:
                    return
                e, fp, q, g_t, is_pair = cmm_q.pop(0)
                last = q == NP - 1
                for half in range(2):
                    hsl = slice(half * 512, (half + 1) * 512)
                    if is_pair:
                        nc.tensor.matmul(
                            psB_cur[(e, fp)][:, hsl],
                            lhsT=cw_sb[e][:, :, q : q + 1],
                            rhs=g_t[:, :, hsl],
                            start=(q == 0),
                            stop=last,
                            perf_mode=DR,
                        )
                    else:
                        nc.tensor.matmul(
                            psB_cur[(e, fp)][:, hsl],
                            lhsT=cw_sb[e][:, 0:1, q : q + 1],
                            rhs=g_t[:, 0, hsl],
                            start=(q == 0),
                            stop=last,
                        )
                if last:
                    finish_pair(e, fp)

            def finish_pair(e, fp):
                # psB [1,1024] f32 -> u8 fp16 cols -> DRAM -> uT columns
                psB = psB_cur.pop((e, fp))
                csl = slice(fp * 1024, (fp + 1) * 1024)
                nc.vector.tensor_copy(u8[e][:, csl], psB[:])
                nc.sync.dma_start(u_d[e : e + 1, csl], u8[e][:, csl])
                nc.sync.dma_start(
                    uT_b[e][:, 8 * fp : 8 * fp + 8],
                    u_d[e, csl].rearrange("(j p) -> p j", p=128),
                )

            def mm1_pair_block(e, fp, w1_t=None):
                """mm1s for one f-chunk pair; one [128,1024] gelu per tt into
                the tt-parity slot of a pair tile; cmms pop with lag-1 so PE
                never waits on ACT."""
                if w1_t is None:
                    w1_t = load_w1(e, fp)
                psB_cur[(e, fp)] = psB_pool.tile([1, 1024], f32, name="psB")
                g_pair = None
                for tt in range(nt):
                    psA = psA_pool.tile([128, 1024], f32, name="psA")
                    tsl = slice(tt * 128, (tt + 1) * 128)
                    for kt in range(NKT):
                        for half in range(2):
                            hsl = slice(half * 512, (half + 1) * 512)
                            rhs = w1_t[:, kt, :].rearrange("p (j n) -> p j n", j=2)[
                                :, :, hsl
                            ]
                            nc.tensor.matmul(
                                psA[:, hsl],
                                lhsT=xg_sb[e][:, kt, :, tsl],
                                rhs=rhs,
                                start=(kt == 0),
                                stop=(kt == NKT - 1) and not has_b1,
                                perf_mode=DR,
                            )
                    if has_b1:
                        for half in range(2):
                            hsl = slice(half * 512, (half + 1) * 512)
                            nc.tensor.matmul(
                                psA[:, hsl],
                                lhsT=ones_sb[:],
                                rhs=b1_sb[e][
                                    :,
                                    fp * 1024 + half * 512 : fp * 1024
                                    + (half + 1) * 512,
                                ],
                                start=False,
                                stop=True,
                            )
                    pop_cmm()  # previous pair's cmms (their gelu is long done)
                    j = tt % 2
                    if j == 0:
                        g_pair = g_pool.tile([128, 2, 1024], fp8, name="gt")
                    nc.scalar.activation(
                        g_pair[:, j, :],
                        psA[:],
                        mybir.ActivationFunctionType.Gelu,
                        scale=1.0 / W1_SCALE,
                    )
                    if j == 1:
                        cmm_q.append((e, fp, tt // 2, g_pair, True))
                    elif tt == nt - 1:
                        cmm_q.append((e, fp, tt // 2, g_pair, False))

            def emit_body():
                w2_tiles.clear()
                # startup: only e0's k0 slice + w1 pair0 k0 gate the first mm
                load_xg(0, 0)
                load_cw(0)
                w1_next = load_w1(0, 0, split=True)
                for kt in range(1, NKT):
                    load_xg(0, kt)
                w2_order = [(e, ft) for e in range(EPC) for ft in range(FT)]
                w2_i = 0
                for e in range(EPC):
                    for fp in range(FCP):
                        if e == 0 and fp < NKT:  # spread e1's input loads out
                            load_xg(1, fp)
                            if fp == 0:
                                load_cw(1)
                        # hoist the NEXT pair's w1 DMA ahead (SP ring order)
                        w1_cur = w1_next
                        if (e, fp) != (EPC - 1, FCP - 1):
                            ne, nf = (e, fp + 1) if fp + 1 < FCP else (e + 1, 0)
                            w1_next = load_w1(ne, nf)
                        # keep the DVE ring fed: 8 w2 tiles per pair-block
                        for _ in range(EPC * FT // (EPC * FCP)):
                            if w2_i < len(w2_order):
                                w2_tiles[w2_order[w2_i]] = w2_prefetch(
                                    *w2_order[w2_i]
                                )
                                w2_i += 1
                        mm1_pair_block(e, fp, w1_t=w1_cur)
                while cmm_q:
                    pop_cmm()
                while w2_i < len(w2_order):
                    w2_tiles[w2_order[w2_i]] = w2_prefetch(*w2_order[w2_i])
                    w2_i += 1

                # ---- phase 2: mm2 tail, 4-way column-tiled ----
                # unit j=(e,half) -> col group 32j, fts 16*half..16*half+15.
                # unit 3 needs the final pair's u: defer its first rounds so
                # units 0-2 bridge the u-transpose latency.
                psC = [
                    psC_pool.tile([128, 512], f32, tag=f"psC{hc}", name=f"psC{hc}")
                    for hc in range(HC)
                ]
                units = [(e, half) for e in range(EPC) for half in range(2)]
                pending = {
                    j: [(r, hc) for r in range(FT // 2) for hc in range(HC)]
                    for j in range(N_UNITS)
                }
                mm2_seq = []
                slot = 0
                while any(pending.values()):
                    for j in range(N_UNITS):
                        if not pending[j]:
                            continue
                        if j == N_UNITS - 1 and slot < 24:
                            continue  # defer last-ready unit ~24 slots
                        mm2_seq.append((j, pending[j].pop(0)))
                        slot += 1
                # start=True clears the whole PSUM bank: only the global-first
                # matmul per bank sets it (stale rows overwrite on first touch
                # since their has_written bits are cleared too)
                first_ix = {}
                last_ix = {}
                for i, (j, (r, hc)) in enumerate(mm2_seq):
                    first_ix.setdefault(hc, i)
                    last_ix[hc] = i
                for i, (j, (r, hc)) in enumerate(mm2_seq):
                    e, half = units[j]
                    ft = 16 * half + r
                    nc.tensor.matmul(
                        psC[hc][32 * j : 32 * j + 1, :],
                        lhsT=uT_b[e][:, ft : ft + 1],
                        rhs=w2_tiles[(e, ft)][:, hc, :],
                        start=(i == first_ix[hc]),
                        stop=(i == last_ix[hc]),
                        tile_position=(0, 32 * j),
                    )

                # copy psC banks to SBUF, ship all 128 rows; host sums rows
                # 0/32/64/96 (others are stale garbage)
                out_sb = small_pool.tile([128, H], f32, tag="out", name="out")
                for hc in range(HC):
                    nc.vector.tensor_copy(
                        out_sb[:, hc * 512 : (hc + 1) * 512], psC[hc][:]
                    )
                nc.sync.dma_start(acc_d[:], out_sb[:])

            if reps > 1:
                # body >256 insts/engine: back-edge branch misses IRAM
                # (~3-4us/iter) without prefetch hints on the big engines
                with tc.For_i(
                    0,
                    reps,
                    1,
                    hint_engines=(mybir.EngineType.PE, mybir.EngineType.SP),
                ):
                    emit_body()
            else:
                emit_body()

    nc.compile()
    _compiled_cache[key] = nc
    return nc


def _prep_inputs(input_tensor, Wg, bg, W1, b1, W2, b2):
    """Host-side gating, top-k, gather, fp8/fp16 conversion. Returns
    (in_maps, nt, has_b1, csum, total_weight)."""
    B, S, _ = input_tensor.shape
    T = B * S
    x = np.ascontiguousarray(input_tensor.reshape(T, H)).astype(np.float32)

    scores = x @ Wg.astype(np.float32) + bg.astype(np.float32)
    order = np.argsort(-scores, axis=1, kind="stable")
    top_i = order[:, :TOP_K]
    top_v = np.take_along_axis(scores, top_i, axis=1).astype(np.float64)
    ex = np.exp(top_v - top_v.max(axis=1, keepdims=True))
    top_w = ex / ex.sum(axis=1, keepdims=True)
    total_weight = float(top_w.sum())

    flat_e = top_i.ravel()
    flat_t = np.repeat(np.arange(T), TOP_K)
    flat_w = top_w.ravel()
    sort = np.argsort(flat_e, kind="stable")
    flat_e, flat_t, flat_w = flat_e[sort], flat_t[sort], flat_w[sort]
    counts = np.bincount(flat_e, minlength=E)
    starts = np.concatenate([[0], np.cumsum(counts)])

    nt = max(1, math.ceil(counts.max() / 128))
    C = nt * 128
    NP = (nt + 1) // 2

    # DoubleRow interleave h = kt*256 + p*2 + j, then fc-pair-major
    xg = np.zeros((E, KH2, 128, 2, C), dtype=FP8)
    w1_c = (W1.reshape(E, KH2, 128, 2, FCP, 1024) * W1_SCALE).astype(FP8)
    w1_c = np.ascontiguousarray(w1_c.transpose(0, 4, 1, 2, 3, 5)).reshape(
        E, FCP, KH2, 128, 2 * 1024
    )
    # cw in device layout [p, j, q] (q padded to 16): slot c=(2q+j)*128+p
    cw = np.zeros((E, 128, 2, 16), dtype=FP8)
    csum = np.zeros(E, dtype=np.float64)
    for e in range(E):
        lo, hi = starts[e], starts[e + 1]
        if hi > lo:
            toks = flat_t[lo:hi]
            xt = x[toks].T
            xg[e, :, :, :, : hi - lo] = xt.astype(FP8).reshape(KH2, 128, 2, hi - lo)
            w_pad = np.zeros(NP * 256, dtype=np.float32)
            w_pad[: hi - lo] = flat_w[lo:hi]
            cw[e, :, :, :NP] = w_pad.reshape(NP, 2, 128).transpose(2, 1, 0).astype(FP8)
            csum[e] = flat_w[lo:hi].sum()

    w2_c = W2.astype(np.float16)
    has_b1 = bool(np.any(b1))

    in_maps = []
    for i in range(N_CORES):
        m = {
            "xg": np.ascontiguousarray(xg[EPC * i : EPC * (i + 1)]),
            "cw": np.ascontiguousarray(cw[EPC * i : EPC * (i + 1)]),
            "w1": np.ascontiguousarray(w1_c[EPC * i : EPC * (i + 1)]),
            "w2": np.ascontiguousarray(w2_c[EPC * i : EPC * (i + 1)]),
        }
        if has_b1:
            m["b1"] = np.ascontiguousarray(
                (b1[EPC * i : EPC * (i + 1)] * W1_SCALE).astype(BF16)
            )
        in_maps.append(m)
    return in_maps, nt, has_b1, csum, total_weight


def _finalize(results, csum, b2, total_weight):
    acc = np.zeros(H, dtype=np.float64)
    for i in range(N_CORES):
        a = results[i]["acc"].astype(np.float64)
        acc += a[0] + a[32] + a[64] + a[96]
    acc += csum @ b2.astype(np.float64)
    return (acc / total_weight).reshape(1, 1, H).astype(np.float32)


def kernel(input_tensor, Wg, bg, W1, b1, W2, b2):
    in_maps, nt, has_b1, csum, total_weight = _prep_inputs(
        input_tensor, Wg, bg, W1, b1, W2, b2
    )
    nc = _build(nt, has_b1)
    res = run_bass_kernel_spmd(nc, in_maps, core_ids=list(range(N_CORES)))
    return _finalize(res.results, csum, b2, total_weight)
